# revision 17
# baseline (speedup 1.0000x reference)
"""Trainium2 Bass kernel for nn_AttentionHead (single-head attention with RoPE + QK-norm).

Contract: kernel(**inputs) takes FULL unsharded inputs
  input_vecs [4, 2048, 1024] f32, qkv_w [3072, 1024] f32, sqk [1024] f32
and returns the FULL output [4, 2048, 1024] f32.

Sharding: 8 cores = (batch b, half z). Core (b, z):
  - projects K and V only for ITS half of the sequence; the pair (2b, 2b+1)
    exchanges K/V via pairwise AllGather, removing projection redundancy;
  - computes attention for batch b's query blocks {0,3} (z=0) or {1,2} (z=1).

v6 dataflow: the K projection is DUPLICATED across the pair (each core
projects K for the full sequence) -- a pairwise collective here costs
~30us trigger lag + ~35us transfer, far more than the +27us of duplicated
matmuls. Only V is pair-split and exchanged (its AllGather hides behind
the Q projection and score phases). With K local, scores and norms never
wait on a collective:
  K proj full (c-outer, 4 key chunks, rope written straight into kt) ->
  K norms (weighted-ones reduce) -> V proj + gather -> Q proj -> q norms
  -> scores-lo -> scores-hi -> transposed denominators -> attnV-hi ->
  attnV-lo.
"""

import numpy as np
import ml_dtypes

D = 1024          # d_model == d_internal
T = 2048
B = 4
HALF = T // 2     # 1024 tokens of K/V per core
ROPE_BASE = 10000.0
NC = 8            # cores
ND = D // 128     # 8 d-tiles
NTC = D // 128    # 8 contraction c-tiles
QB = 512          # query block size
NKLO, NKHI = 8, 16  # k-tiles processed for chunk-lo / chunk-hi (uniform program)

BF16 = ml_dtypes.bfloat16

# ---------------------------------------------------------------------------
# Infra patch: this walrus build supports only ONE sync-wait per instruction.
# Tile attaches multiple; split the extras onto NoOps inserted just before.
# ---------------------------------------------------------------------------
_PATCHED = False


def _install_patches():
    global _PATCHED
    if _PATCHED:
        return
    _PATCHED = True
    import json as _json
    import concourse.bass as _bass

    orig = _bass.Bass.to_json_bytes

    def _split_waits(m):
        ctr = 0
        for fn in m.get("functions", []):
            for blk in fn.get("blocks", []):
                insts = blk.get("instructions")
                if not insts:
                    continue
                out = []
                changed = False
                for inst in insts:
                    si = inst.get("sync_info")
                    w = (si or {}).get("on_wait") or []
                    if len(w) > 1:
                        changed = True
                        for j in range(len(w) - 1):
                            ctr += 1
                            out.append({
                                "name": f"waitsplit-{ctr}-{inst['name']}",
                                "engine": inst["engine"],
                                "opcode": "NoOp",
                                "ins": [],
                                "outs": [],
                                "sync_info": {"on_wait": [w[j]], "on_update": []},
                            })
                        si["on_wait"] = [w[-1]]
                    out.append(inst)
                if changed:
                    blk["instructions"] = out
        return m, ctr

    def to_json_bytes(self):
        raw = orig(self)
        m = _json.loads(raw)
        m, n = _split_waits(m)
        if n:
            raw = _json.dumps(m).encode()
        return raw

    _bass.Bass.to_json_bytes = to_json_bytes


# ---------------------------------------------------------------------------
# Bass program (identical for all 8 cores; per-core behavior comes from data)
# ---------------------------------------------------------------------------
_PROGRAM = None
_GROUPS = [[0, 1], [2, 3], [4, 5], [6, 7]]


def _build_program():
    import concourse.bass as bass
    import concourse.mybir as mybir
    from concourse.tile import TileContext

    BF = mybir.dt.bfloat16
    F32 = mybir.dt.float32
    AF = mybir.ActivationFunctionType
    OP = mybir.AluOpType

    nc = bass.Bass(num_devices=NC)

    # ---- I/O ----
    xlo_d = nc.dram_tensor("xlo", [D, HALF], BF, kind="ExternalInput")
    xhi_d = nc.dram_tensor("xhi", [D, HALF], BF, kind="ExternalInput")
    xh_d = nc.dram_tensor("xh", [D, HALF], BF, kind="ExternalInput")
    xq_d = nc.dram_tensor("xq", [D, 2 * QB], BF, kind="ExternalInput")
    wt_d = nc.dram_tensor("wt", [D, 3 * D], BF, kind="ExternalInput")
    cosf_d = nc.dram_tensor("cosf", [D // 2, T], BF, kind="ExternalInput")
    sinf_d = nc.dram_tensor("sinf", [D // 2, T], BF, kind="ExternalInput")
    cosq_d = nc.dram_tensor("cosq", [D // 2, 2 * QB], BF, kind="ExternalInput")
    sinq_d = nc.dram_tensor("sinq", [D // 2, 2 * QB], BF, kind="ExternalInput")
    s2_d = nc.dram_tensor("s2", [128, ND], F32, kind="ExternalInput")
    si_d = nc.dram_tensor("s2i2", [128, ND], BF, kind="ExternalInput")
    mlo_d = nc.dram_tensor("masklo", [NKLO, 128, QB], BF, kind="ExternalInput")
    mhi_d = nc.dram_tensor("maskhi", [NKHI - NKLO, 128, QB], BF, kind="ExternalInput")
    out_d = nc.dram_tensor("out", [2 * QB, D], F32, kind="ExternalOutput")
    vh_d = nc.dram_tensor("vhalf", [HALF, D], BF, kind="Internal")
    vg_d = nc.dram_tensor("vgath", [T, D], BF, kind="Internal")
    nrm_d = nc.dram_tensor("nrmscr", [NKHI, 128], F32, kind="Internal")
    den_d = nc.dram_tensor("denscr", [8, 128], F32, kind="Internal")

    with TileContext(nc) as tc:
        with tc.tile_pool(name="persist", bufs=1) as pp:
            kt = pp.tile([128, ND * T], BF, tag="kt")             # 32K
            vt = pp.tile([128, 16 * D], BF, tag="vt")             # 32K
            qt = pp.tile([128, ND * 2 * QB], BF, tag="qt")        # 16K
            rnk = pp.tile([128, 16], F32, tag="rnk")
            rden = pp.tile([128, 8], F32, tag="rden")
            s2 = pp.tile([128, ND], F32, tag="s2")
            s2i2 = pp.tile([128, ND], BF, tag="s2i2")
            ones_bf = pp.tile([128, 1], BF, tag="ones_bf")
            ones1x = pp.tile([1, 128], F32, tag="ones1x")

            nc.scalar.dma_start(s2[:], s2_d[:, :])
            nc.scalar.dma_start(s2i2[:], si_d[:, :])
            nc.vector.memset(ones_bf[:], 1.0)
            nc.vector.memset(ones1x[:], 1.0)

            with tc.tile_pool(name="pw", bufs=1) as pw:
                wk = [pw.tile([128, D], BF, tag=f"wk{c}", name=f"wk{c}") for c in range(NTC)]
                wv = [pw.tile([128, D], BF, tag=f"wv{c}", name=f"wv{c}") for c in range(NTC)]
                wq = [pw.tile([128, D], BF, tag=f"wq{c}", name=f"wq{c}") for c in range(NTC)]

                # gpsimd queue: weights; sync queue: x columns (parallel)
                for c in range(NTC):
                    nc.gpsimd.dma_start(wk[c][:], wt_d[c * 128:(c + 1) * 128, D:2 * D])
                for c in range(NTC):
                    nc.gpsimd.dma_start(wv[c][:], wt_d[c * 128:(c + 1) * 128, 2 * D:3 * D])
                for c in range(NTC):
                    nc.gpsimd.dma_start(wq[c][:], wt_d[c * 128:(c + 1) * 128, 0:D])

                with tc.tile_pool(name="pxk", bufs=1) as pxk, \
                     tc.tile_pool(name="pxh", bufs=1) as pxh:
                    xk = [pxk.tile([128, T], BF, tag=f"xk{c}", name=f"xk{c}") for c in range(NTC)]
                    xh = [pxh.tile([128, HALF], BF, tag=f"xh{c}", name=f"xh{c}") for c in range(NTC)]
                    for c in range(NTC):
                        nc.sync.dma_start(xk[c][:, 0:HALF], xlo_d[c * 128:(c + 1) * 128, :])
                        nc.sync.dma_start(xk[c][:, HALF:T], xhi_d[c * 128:(c + 1) * 128, :])
                    for c in range(NTC):
                        nc.sync.dma_start(xh[c][:], xh_d[c * 128:(c + 1) * 128, :])

                    # ---- P1: K projection over the FULL sequence (4 chunks) ----
                    with tc.tile_pool(name="pks", bufs=1) as pks:
                        with tc.tile_pool(name="psk", bufs=1, space="PSUM") as psk:
                            for g in range(4):
                                k0 = g * QB
                                cosc = pks.tile([128, 4 * QB], BF, tag="cosc", name="cosc", bufs=1)
                                sinc = pks.tile([128, 4 * QB], BF, tag="sinc", name="sinc", bufs=1)
                                for i in range(4):
                                    nc.scalar.dma_start(cosc[:, i * QB:(i + 1) * QB], cosf_d[i * 128:(i + 1) * 128, k0:k0 + QB])
                                    nc.scalar.dma_start(sinc[:, i * QB:(i + 1) * QB], sinf_d[i * 128:(i + 1) * 128, k0:k0 + QB])
                                pkt = [psk.tile([128, QB], F32, tag=f"pk{dt}", name=f"pk{dt}") for dt in range(ND)]
                                for c in range(NTC):
                                    for dt in range(ND):
                                        nc.tensor.matmul(pkt[dt][:], wk[c][:, dt * 128:(dt + 1) * 128],
                                                         xk[c][:, k0:k0 + QB],
                                                         start=(c == 0), stop=(c == NTC - 1))
                                kms = []
                                for dt in range(ND):
                                    km = pks.tile([128, QB], BF, tag=f"km{dt}", name=f"km{dt}", bufs=1)
                                    nc.scalar.copy(km[:], pkt[dt][:])
                                    kms.append(km)
                                # rope with fused s2 scale, writing straight into kt
                                for j in range(4):
                                    km_a, km_b = kms[j], kms[j + 4]
                                    ca = cosc[:, j * QB:(j + 1) * QB]
                                    sa = sinc[:, j * QB:(j + 1) * QB]
                                    t_a = pks.tile([128, QB], BF, tag="kra", name="t_a", bufs=1)
                                    t_b = pks.tile([128, QB], BF, tag="krb", name="t_b", bufs=1)
                                    nc.vector.scalar_tensor_tensor(t_a[:], km_a[:], s2[:, j:j + 1], ca, op0=OP.mult, op1=OP.mult)
                                    nc.vector.scalar_tensor_tensor(t_b[:], km_b[:], s2[:, j:j + 1], sa, op0=OP.mult, op1=OP.mult)
                                    nc.vector.tensor_sub(kt[:, j * T + k0: j * T + k0 + QB], t_a[:], t_b[:])
                                    t_c = pks.tile([128, QB], BF, tag="kra", name="t_c", bufs=1)
                                    t_e = pks.tile([128, QB], BF, tag="krb", name="t_e", bufs=1)
                                    nc.vector.scalar_tensor_tensor(t_c[:], km_b[:], s2[:, j + 4:j + 5], ca, op0=OP.mult, op1=OP.mult)
                                    nc.vector.scalar_tensor_tensor(t_e[:], km_a[:], s2[:, j + 4:j + 5], sa, op0=OP.mult, op1=OP.mult)
                                    nc.vector.tensor_add(kt[:, (j + 4) * T + k0: (j + 4) * T + k0 + QB], t_c[:], t_e[:])

                        # K row norms from local kt (weighted-ones reduce)
                        nrm_sb = pks.tile([1, 1024], F32, tag="nrm_sb")
                        rnkn2 = pks.tile([128, 16], F32, tag="rnkn2")
                        with tc.tile_pool(name="psn", bufs=1, space="PSUM") as psn:
                            for g in range(4):
                                pn = psn.tile([1, QB], F32, tag="pn", name="pn", bufs=1)
                                for dt in range(ND):
                                    ksq = pks.tile([128, QB], BF, tag="ksq", name="ksq", bufs=4)
                                    nc.vector.tensor_tensor(ksq[:], kt[:, dt * T + g * QB: dt * T + (g + 1) * QB],
                                                            kt[:, dt * T + g * QB: dt * T + (g + 1) * QB], op=OP.mult)
                                    nc.tensor.matmul(pn[:], s2i2[:, dt:dt + 1], ksq[:],
                                                     start=(dt == 0), stop=(dt == ND - 1))
                                nc.scalar.copy(nrm_sb[:, (g % 2) * QB:(g % 2 + 1) * QB], pn[:])
                                if g % 2 == 1:
                                    nc.scalar.dma_start(nrm_d[(g - 1) * 4:(g + 1) * 4, :], nrm_sb[:, :])
                        for c in range(NKHI):
                            nc.sync.dma_start(rnkn2[:, c:c + 1], nrm_d[c:c + 1, :])
                        nc.scalar.activation(rnkn2[:], rnkn2[:], AF.Sqrt)
                        nc.vector.reciprocal(rnk[:], rnkn2[:])

                    # ---- P2: V projection; staging DMAs ride the scalar queue ----
                    with tc.tile_pool(name="pvs", bufs=1) as pvs:
                        with tc.tile_pool(name="psv", bufs=1, space="PSUM") as psv:
                            for tt in range(HALF // 128):
                                vst = pvs.tile([128, D], BF, tag="vst", name="vst", bufs=3)
                                for dch in range(2):
                                    p = psv.tile([128, 512], F32, tag="pv", name="pv", bufs=4)
                                    for c in range(NTC):
                                        nc.tensor.matmul(p[:], xh[c][:, tt * 128:(tt + 1) * 128],
                                                         wv[c][:, dch * 512:(dch + 1) * 512],
                                                         start=(c == 0), stop=(c == NTC - 1))
                                    nc.scalar.copy(vst[:, dch * 512:(dch + 1) * 512], p[:])
                                nc.scalar.dma_start(vh_d[tt * 128:(tt + 1) * 128, :], vst[:])
                        nc.gpsimd.collective_compute(
                            kind="AllGather", op=OP.bypass, replica_groups=_GROUPS,
                            ins=[vh_d[:, :]], outs=[vg_d[:, :]])

                # ---- P3: Q proj ch0 -> K norms -> Q proj ch1 -> q norms + rope ----
                with tc.tile_pool(name="pqs", bufs=1) as pqs:
                    xq = pqs.tile([128, NTC * 2 * QB], BF, tag="xq")      # 16K
                    cosq = pqs.tile([128, 4 * 2 * QB], BF, tag="cosq")    # 8K
                    sinq = pqs.tile([128, 4 * 2 * QB], BF, tag="sinq")    # 8K
                    for c in range(NTC):
                        nc.gpsimd.dma_start(xq[:, c * 2 * QB:(c + 1) * 2 * QB], xq_d[c * 128:(c + 1) * 128, :])
                    for i in range(4):
                        nc.sync.dma_start(cosq[:, i * 2 * QB:(i + 1) * 2 * QB], cosq_d[i * 128:(i + 1) * 128, :])
                        nc.sync.dma_start(sinq[:, i * 2 * QB:(i + 1) * 2 * QB], sinq_d[i * 128:(i + 1) * 128, :])

                    with tc.tile_pool(name="psq", bufs=1, space="PSUM") as psq, \
                         tc.tile_pool(name="psnq", bufs=1, space="PSUM") as psnq:
                        qtmps = []

                        def qproj(ch):
                            q0 = ch * QB
                            qtmp = pqs.tile([128, ND * QB], BF, tag="qtmp", name="qtmp", bufs=2)
                            for i in range(ND):
                                p = psq.tile([128, QB], F32, tag="pq", name="pq", bufs=4)
                                for c in range(NTC):
                                    nc.tensor.matmul(p[:], wq[c][:, i * 128:(i + 1) * 128],
                                                     xq[:, c * 2 * QB + q0: c * 2 * QB + q0 + QB],
                                                     start=(c == 0), stop=(c == NTC - 1))
                                nc.scalar.copy(qtmp[:, i * QB:(i + 1) * QB], p[:])
                            qtmps.append(qtmp)

                        qproj(0)
                        qproj(1)

                        for ch in range(2):
                            q0 = ch * QB
                            qtmp = qtmps[ch]
                            pnq = psnq.tile([1, QB], F32, tag="pnq", name="pnq", bufs=1)
                            for i in range(ND):
                                sqq = pqs.tile([128, QB], BF, tag="sqq", name="sqq", bufs=2)
                                nc.vector.tensor_tensor(sqq[:], qtmp[:, i * QB:(i + 1) * QB],
                                                        qtmp[:, i * QB:(i + 1) * QB], op=OP.mult)
                                nc.tensor.matmul(pnq[:], ones_bf[:], sqq[:], start=(i == 0), stop=(i == ND - 1))
                            rnq = pqs.tile([1, QB], F32, tag="rnq", name="rnq", bufs=2)
                            nc.scalar.activation(rnq[:], pnq[:], AF.Sqrt)
                            nc.vector.reciprocal(rnq[:], rnq[:])
                            pbc = psnq.tile([128, QB], F32, tag="pbc", name="pbc", bufs=1)
                            nc.tensor.matmul(pbc[:], ones1x[:], rnq[:], start=True, stop=True)
                            pbc_bf = pqs.tile([128, QB], BF, tag="pbc_bf", name="pbc_bf", bufs=2)
                            nc.scalar.copy(pbc_bf[:], pbc[:])
                            for i in range(4):
                                ca = cosq[:, i * 2 * QB + q0: i * 2 * QB + q0 + QB]
                                sa = sinq[:, i * 2 * QB + q0: i * 2 * QB + q0 + QB]
                                lo = qtmp[:, i * QB:(i + 1) * QB]
                                hi = qtmp[:, (i + 4) * QB:(i + 5) * QB]
                                t_a = pqs.tile([128, QB], BF, tag="ropea", name="t_a", bufs=2)
                                t_b = pqs.tile([128, QB], BF, tag="ropeb", name="t_b", bufs=2)
                                nc.vector.tensor_tensor(t_a[:], lo, ca, op=OP.mult)
                                nc.vector.tensor_tensor(t_b[:], hi, sa, op=OP.mult)
                                nc.vector.tensor_sub(t_a[:], t_a[:], t_b[:])
                                nc.vector.tensor_tensor(qt[:, i * 2 * QB + q0: i * 2 * QB + q0 + QB],
                                                        t_a[:], pbc_bf[:], op=OP.mult)
                                t_c = pqs.tile([128, QB], BF, tag="ropea", name="t_c", bufs=2)
                                t_e = pqs.tile([128, QB], BF, tag="ropeb", name="t_e", bufs=2)
                                nc.vector.tensor_tensor(t_c[:], hi, ca, op=OP.mult)
                                nc.vector.tensor_tensor(t_e[:], lo, sa, op=OP.mult)
                                nc.vector.tensor_add(t_c[:], t_c[:], t_e[:])
                                nc.vector.tensor_tensor(qt[:, (i + 4) * 2 * QB + q0: (i + 4) * 2 * QB + q0 + QB],
                                                        t_c[:], pbc_bf[:], op=OP.mult)

            # V reload prefetch (sync, waits on the V-gather semaphore)
            for kti in range(16):
                nc.sync.dma_start(vt[:, kti * D:(kti + 1) * D], vg_d[kti * 128:(kti + 1) * 128, :])

            # ---- P4: attention ----
            with tc.tile_pool(name="patt", bufs=1) as pat:
                ex_lo = pat.tile([128, NKLO * QB], BF, tag="ex_lo")
                ex_hi = pat.tile([128, NKHI * QB], BF, tag="ex_hi")
                den_sb = pat.tile([1, 1024], F32, tag="den_sb")
                rdn2 = pat.tile([128, 8], F32, tag="rdn2")
                exs = (ex_lo, ex_hi)
                with tc.tile_pool(name="pss", bufs=1, space="PSUM") as pss, \
                     tc.tile_pool(name="psd", bufs=1, space="PSUM") as psd:
                    for ch, (n_k, mask_d, mask_start) in enumerate(
                            ((NKLO, mlo_d, 0), (NKHI, mhi_d, NKLO))):
                        q0 = ch * QB
                        ex = exs[ch]
                        for kti in range(n_k):
                            ps_s = pss.tile([128, QB], F32, tag="pscore", name="ps_s", bufs=3)
                            for i in range(ND):
                                nc.tensor.matmul(ps_s[:], kt[:, i * T + kti * 128: i * T + (kti + 1) * 128],
                                                 qt[:, i * 2 * QB + q0: i * 2 * QB + q0 + QB],
                                                 start=(i == 0), stop=(i == ND - 1))
                            exsl = ex[:, kti * QB:(kti + 1) * QB]
                            nc.scalar.activation(exsl, ps_s[:], AF.Exp, bias=0.0, scale=rnk[:, kti:kti + 1])
                            if kti >= mask_start:
                                mt = pat.tile([128, QB], BF, tag="mask", name="mt", bufs=3)
                                nc.gpsimd.dma_start(mt[:], mask_d[kti - mask_start, :, :])
                                nc.vector.tensor_tensor(exsl, exsl, mt[:], op=OP.mult)
                    # transposed denominators: [1, 512] per chunk, 512-row matmuls
                    for ch, n_k in enumerate((NKLO, NKHI)):
                        ex = exs[ch]
                        pd = psd.tile([1, QB], F32, tag=f"pd{ch}", name=f"pd{ch}")
                        for kti in range(n_k):
                            nc.tensor.matmul(pd[:], ones_bf[:], ex[:, kti * QB:(kti + 1) * QB],
                                             start=(kti == 0), stop=(kti == n_k - 1))
                        nc.scalar.copy(den_sb[:, ch * QB:(ch + 1) * QB], pd[:])
                    nc.scalar.dma_start(den_d[:, :], den_sb[:, :])
                    for c in range(8):
                        nc.sync.dma_start(rdn2[:, c:c + 1], den_d[c:c + 1, :])
                    nc.vector.reciprocal(rden[:], rdn2[:])

                with tc.tile_pool(name="pso", bufs=1, space="PSUM") as pso:
                    for ch, n_k in ((1, NKHI), (0, NKLO)):
                        q0 = ch * QB
                        ex = exs[ch]
                        for sp in range(2):
                            poa = pso.tile([128, 512], F32, tag="pout0", name="poa", bufs=2)
                            pob = pso.tile([128, 512], F32, tag="pout1", name="pob", bufs=2)
                            poc = pso.tile([128, 512], F32, tag="pout2", name="poc", bufs=2)
                            pod = pso.tile([128, 512], F32, tag="pout3", name="pod", bufs=2)
                            po = ((poa, pob), (poc, pod))
                            for kti in range(n_k):
                                for s01 in range(2):
                                    sub = sp * 2 + s01
                                    for dch in range(2):
                                        nc.tensor.matmul(po[s01][dch][:],
                                                         ex[:, kti * QB + sub * 128: kti * QB + (sub + 1) * 128],
                                                         vt[:, kti * D + dch * 512: kti * D + (dch + 1) * 512],
                                                         start=(kti == 0), stop=(kti == n_k - 1))
                            for s01 in range(2):
                                sub = sp * 2 + s01
                                ot = pat.tile([128, D], F32, tag="ot", name="ot", bufs=4)
                                rsc = rden[:, ch * 4 + sub:ch * 4 + sub + 1]
                                for dch in range(2):
                                    nc.scalar.activation(ot[:, dch * 512:(dch + 1) * 512],
                                                         po[s01][dch][:], AF.Copy, bias=0.0, scale=rsc)
                                if s01 == 0:
                                    nc.sync.dma_start(out_d[q0 + sub * 128: q0 + (sub + 1) * 128, :], ot[:])
                                else:
                                    nc.gpsimd.dma_start(out_d[q0 + sub * 128: q0 + (sub + 1) * 128, :], ot[:])

    return nc


def _get_program():
    global _PROGRAM
    if _PROGRAM is None:
        _install_patches()
        _PROGRAM = _build_program()
    return _PROGRAM


# ---------------------------------------------------------------------------
# Host-side prep + launch
# ---------------------------------------------------------------------------
def _rope_tables():
    inv_freq = (1.0 / (ROPE_BASE ** (np.arange(0, D, 2, dtype=np.float32) / D))).astype(np.float32)
    t = np.arange(T, dtype=np.float32)
    freqs = t[:, None] * inv_freq[None, :]          # [T, 512]
    cos = np.cos(freqs).T.copy()                    # [512, T]
    sin = np.sin(freqs).T.copy()
    return cos, sin


def _mask_tiles(block, kt_lo, kt_hi):
    """[kt_hi-kt_lo, 128, 512] 0/1: allowed = key_global <= query_global."""
    n = kt_hi - kt_lo
    m = np.zeros((n, 128, QB), dtype=np.float32)
    qg = block * QB + np.arange(QB)[None, :]
    for idx, kti in enumerate(range(kt_lo, kt_hi)):
        kg = kti * 128 + np.arange(128)[:, None]
        m[idx] = (kg <= qg).astype(np.float32)
    return m


# kept for test.py introspection
LAST_RESULT = None


def kernel(input_vecs, qkv_w, sqk, _trace=False):
    global LAST_RESULT
    _install_patches()
    from concourse.bass_utils import run_bass_kernel_spmd

    nc = _get_program()

    f32 = np.float32
    x = np.asarray(input_vecs, f32)
    w = np.asarray(qkv_w, f32)
    s = np.asarray(sqk, f32)

    wt_bf = np.ascontiguousarray(w.T).astype(BF16)                  # [1024, 3072]
    sqk_eff = s * np.sqrt(np.float32(D)).astype(f32)
    s2 = (np.sqrt(np.float32(D)).astype(f32) * sqk_eff * sqk_eff).astype(f32)   # [1024]
    s2_t = np.ascontiguousarray(s2.reshape(ND, 128).T)              # [128, 8]: [p,j] = s2[j*128+p]
    s2i2_t = np.ascontiguousarray((1.0 / (s2 * s2)).reshape(ND, 128).T).astype(BF16)
    cos, sin = _rope_tables()
    cos_bf = np.ascontiguousarray(cos).astype(BF16)
    sin_bf = np.ascontiguousarray(sin).astype(BF16)

    in_maps = []
    metas = []
    for c in range(NC):
        b, z = c // 2, c % 2
        blo, bhi = (0, 3) if z == 0 else (1, 2)
        xt = np.ascontiguousarray(x[b].T)                           # [1024, 2048] f32
        qcols = np.concatenate([xt[:, blo * QB:(blo + 1) * QB],
                                xt[:, bhi * QB:(bhi + 1) * QB]], axis=1)
        cosq = np.concatenate([cos[:, blo * QB:(blo + 1) * QB],
                               cos[:, bhi * QB:(bhi + 1) * QB]], axis=1)
        sinq = np.concatenate([sin[:, blo * QB:(blo + 1) * QB],
                               sin[:, bhi * QB:(bhi + 1) * QB]], axis=1)
        h0 = z * HALF
        in_maps.append({
            "xlo": np.ascontiguousarray(xt[:, 0:HALF]).astype(BF16),
            "xhi": np.ascontiguousarray(xt[:, HALF:T]).astype(BF16),
            "xh": np.ascontiguousarray(xt[:, h0:h0 + HALF]).astype(BF16),
            "xq": np.ascontiguousarray(qcols).astype(BF16),
            "wt": wt_bf,
            "cosf": cos_bf,
            "sinf": sin_bf,
            "cosq": np.ascontiguousarray(cosq).astype(BF16),
            "sinq": np.ascontiguousarray(sinq).astype(BF16),
            "s2": s2_t,
            "s2i2": s2i2_t,
            "masklo": _mask_tiles(blo, 0, NKLO).astype(BF16),
            "maskhi": _mask_tiles(bhi, NKLO, NKHI).astype(BF16),
        })
        metas.append((b, blo, bhi))

    res = run_bass_kernel_spmd(nc, in_maps, core_ids=list(range(NC)), trace=_trace)
    LAST_RESULT = res

    out = np.empty((B, T, D), dtype=f32)
    for c, (b, blo, bhi) in enumerate(metas):
        o = np.asarray(res.results[c]["out"], f32)
        out[b, blo * QB:(blo + 1) * QB] = o[:QB]
        out[b, bhi * QB:(bhi + 1) * QB] = o[QB:]
    return out
